# revision 51
# baseline (speedup 1.0000x reference)
"""Multi-head causal self-attention with RoPE on 8 Trainium2 NeuronCores.

Sharding: 12 heads over 8 cores. Core pairs (2p, 2p+1) share 3 heads:
  core 2p:   slot A = head 3p   (all 8 q-blocks), slot B = head 3p+1, q-blocks BSET_EVEN
  core 2p+1: slot A = head 3p+2 (all 8 q-blocks), slot B = head 3p+1, q-blocks BSET_ODD
Every core: 2 heads on 128 partitions, balanced causal cost (both bsets have
equal causal area; the pair splits front-heavy qb0 / tail-heavy qb7 across the
two programs). Two NEFFs dispatched concurrently on device groups [0..3]/[4..7].

v2 (bf16): all matmuls bf16 (same PE rate as fp32r at 512-wide, full rate at
narrow widths). Causal mask folded into the logits PSUM accumulation via an
eye @ M matmul (M = -200 upper triangle; exp(-200*0.125) == 0 for our sums,
and unlike -1e30 it does not NaN the hardware ACT exp table), so no separate
masking pass is needed. AV is restructured as out[q,65] = ex^T @ [V|1] per
(q-tile, k-tile) pair: the scores tile is the stationary operand, so each
accumulation step costs only 65 PE rows instead of 512. NOTE: matmul
start=True clears has_written for the WHOLE PSUM bank, so only the first AV
matmul per slot-bank per q-block sets it. Softmax division is a per-partition
tensor_scalar on DVE (Pool cannot access PSUM) in the [q, dh] layout, then
the result is transposed back to [dh, q] for the O-projection. RoPE pairs are
(d, d+16) within each 32-partition quadrant (host-side weight-row permutation)
so the pair swap is one stream_shuffle rotate-16.

Scheduling: projection/O-projection work is queued as fine-grained background
steps drained between attention groups (deadline-forced before the logits that
need them) so the ACT engine -- the binding resource, ~0.83 ns/col for the
~13M-element causal exp -- stays fed; per-slot AV emission interleaves with
the next group's logits to hide the exp->logits PSUM-slot ping-pong.
"""
import sys, os
sys.path.insert(0, "/opt/trn_rl_repo")
os.environ.setdefault("MYCRO_LOCAL_CACHE", "1")

import numpy as np

S, D, H, DH = 4096, 768, 12, 64
NCH, CH = 8, 512     # token chunks (projection phase)
NQB, QB = 8, 512     # query blocks
NKT, KT = 32, 128    # key tiles
VPW = 130            # vp_all per-ktile width: [V_A(64) | 1 | V_B(64) | 1]
THETA = 10000.0
ROT16 = [(i + 16) % 32 for i in range(32)]
AVS = 512            # av psum tile: slot s qtile j at col s*AVS + j*65

BSET_EVEN = (1, 2, 4, 7)
BSET_ODD = (0, 3, 5, 6)

# core -> (headA, headB)
CORE_HEADS = []
for p in range(4):
    CORE_HEADS.append((3 * p, 3 * p + 1))
    CORE_HEADS.append((3 * p + 2, 3 * p + 1))

# row r (0..63) inside a head slot -> original within-head dim.
# quadrant q = r//32, i = r%32: freq f = 16*q + (i%16); i<16 -> dim 2f, else 2f+1.
PERM64 = []
for r in range(64):
    q, i = r // 32, r % 32
    f = 16 * q + (i % 16)
    PERM64.append(2 * f if i < 16 else 2 * f + 1)
PERM64 = np.array(PERM64)

_PROGRAMS = {}


def _build_program(bset):
    import concourse.bass as bass
    import concourse.tile as tile
    from concourse import bacc, mybir
    from concourse.alu_op_type import AluOpType

    dt = mybir.dt
    F32, BF16, F16 = dt.float32, dt.bfloat16, dt.float16
    AF = mybir.ActivationFunctionType

    nc = bacc.Bacc("TRN2", target_bir_lowering=False, debug=False, num_devices=4)

    xt_d = nc.dram_tensor("xt", [D, S], BF16, kind="ExternalInput").ap()
    wqt_d = nc.dram_tensor("wqt", [D, 128], BF16, kind="ExternalInput").ap()
    wkt_d = nc.dram_tensor("wkt", [D, 128], BF16, kind="ExternalInput").ap()
    wvt_d = nc.dram_tensor("wvt", [D, 128], BF16, kind="ExternalInput").ap()
    wot_d = nc.dram_tensor("wot", [128, D], BF16, kind="ExternalInput").ap()
    cosf_d = nc.dram_tensor("cosf", [128, S], F32, kind="ExternalInput").ap()
    sins_d = nc.dram_tensor("sins", [128, S], F32, kind="ExternalInput").ap()
    maskm_d = nc.dram_tensor("maskm", [128, 128], BF16, kind="ExternalInput").ap()
    eye_d = nc.dram_tensor("eye", [128, 128], BF16, kind="ExternalInput").ap()
    opart_d = nc.dram_tensor("opart", [D, S], F16, kind="ExternalOutput").ap()
    DEBUG = bool(os.environ.get("MHA_DEBUG"))
    if DEBUG:
        dbg_kt = nc.dram_tensor("dbg_kt", [128, S], BF16, kind="ExternalOutput").ap()
        dbg_qt = nc.dram_tensor("dbg_qt", [128, S], BF16, kind="ExternalOutput").ap()
        dbg_vp = nc.dram_tensor("dbg_vp", [128, NKT * VPW], BF16, kind="ExternalOutput").ap()
        dbg_at = nc.dram_tensor("dbg_at", [128, S], BF16, kind="ExternalOutput").ap()

    with tile.TileContext(nc) as tc:
        with (
            tc.tile_pool(name="const", bufs=1) as cp,
            tc.tile_pool(name="xc", bufs=2) as xcp,
            tc.tile_pool(name="rt", bufs=2) as rtp,
            tc.tile_pool(name="ex", bufs=4) as exp_pool,
            tc.tile_pool(name="avs", bufs=6) as avsb,
            tc.tile_pool(name="osb", bufs=2) as osb,
            tc.tile_pool(name="psJ", bufs=2, space="PSUM") as psJ,
            tc.tile_pool(name="psL", bufs=2, space="PSUM") as psL,
            tc.tile_pool(name="psAV", bufs=1, space="PSUM") as psAV,
        ):
            kt_rot = cp.tile([128, S], BF16, tag="ktrot")
            qt_rot = cp.tile([128, S], BF16, tag="qtrot")
            vp_all = cp.tile([128, NKT * VPW], BF16, tag="vpall")
            at_all = cp.tile([128, S], BF16, tag="atall")
            maskm = cp.tile([128, 128], BF16, tag="maskm")
            eye = cp.tile([128, 128], BF16, tag="eye")
            wo_all = cp.tile([128, D], BF16, tag="wo")
            wq_all = cp.tile([128, 6 * 128], BF16, tag="wqa")
            wk_all = cp.tile([128, 6 * 128], BF16, tag="wka")
            wv_all = cp.tile([128, 6 * 128], BF16, tag="wva")
            ones_sb = cp.tile([128, 64], BF16, tag="ones")
            wq_t = [wq_all[:, i * 128:(i + 1) * 128] for i in range(6)]
            wk_t = [wk_all[:, i * 128:(i + 1) * 128] for i in range(6)]
            wv_t = [wv_all[:, i * 128:(i + 1) * 128] for i in range(6)]

            nc.vector.memset(ones_sb[:], 1.0)

            def dma_w(wall, wd):
                nc.sync.dma_start(
                    wall[:].rearrange("p (i c) -> p i c", c=128),
                    wd[:].rearrange("(i p) c -> p i c", p=128))

            def init_consts_late():
                # issued after chunk-0's DMAs so the K projection starts early
                dma_w(wq_all, wqt_d)
                nc.sync.dma_start(maskm[:], maskm_d[:])
                nc.sync.dma_start(eye[:], eye_d[:])
                dma_w(wv_all, wvt_d)
                nc.sync.dma_start(wo_all[:], wot_d[:])
                # ones columns of vp_all: cols {130t+64, 130t+129}
                nc.vector.tensor_copy(
                    vp_all[:].rearrange(
                        "p (t x) -> p t x", x=VPW)[:, :, 64:VPW:65],
                    ones_sb[:].rearrange("p (t x) -> p t x", x=2))

            dma_w(wk_all, wkt_d)

            # ---------------- attention -------------------------
            NEVER = NCH + 1  # deadline for steps with no ordering constraint

            def queue_oproj(c):
                contr = 128 if c in bset else 64
                box = {}

                def step(mt):
                    if mt == 0:
                        box["ot"] = osb.tile([128, 6 * CH], F16, tag="ot",
                                             name=f"ot{c}")
                    po = psJ.tile([128, CH], F32, tag="pj", name=f"po{c}_{mt}")
                    nc.tensor.matmul(
                        po[:], wo_all[0:contr, mt * 128:(mt + 1) * 128],
                        at_all[0:contr, c * CH:(c + 1) * CH],
                        start=True, stop=True)
                    # Pool cannot read PSUM: stage on ACT only in the early
                    # chunks (ACT idles in the proj-heavy front), DVE after
                    if c <= 2 and mt % 2 == 0:
                        nc.scalar.copy(box["ot"][:, mt * CH:(mt + 1) * CH],
                                       po[:])
                    else:
                        nc.vector.tensor_copy(
                            box["ot"][:, mt * CH:(mt + 1) * CH], po[:])
                    if mt == 5:
                        nc.sync.dma_start(
                            opart_d[:, c * CH:(c + 1) * CH].rearrange(
                                "(mt p) c -> p mt c", p=128),
                            box["ot"][:].rearrange("p (mt c) -> p mt c", c=CH))

                for mt in range(6):
                    bg_steps.append((NEVER, (lambda m: lambda: step(m))(mt)))

            bg_steps = []  # deferred projection work, drained between groups

            def emit_oproj_span(c, j0, j1, tag2):
                # O-projection over query columns [c*CH + 128*j0, c*CH + 128*j1)
                contr = 128 if c in bset else 64
                w = 128 * (j1 - j0)
                c0 = c * CH + 128 * j0
                ot = osb.tile([128, 6 * w], F16, tag="ot", name=f"ot{c}_{tag2}")
                for mt in range(6):
                    po = psJ.tile([128, w], F32, tag="pj",
                                  name=f"po{c}_{tag2}_{mt}")
                    nc.tensor.matmul(
                        po[:], wo_all[0:contr, mt * 128:(mt + 1) * 128],
                        at_all[0:contr, c0:c0 + w],
                        start=True, stop=True)
                    if mt % 2 == 0:
                        nc.scalar.copy(ot[:, mt * w:(mt + 1) * w], po[:])
                    else:
                        nc.vector.tensor_copy(ot[:, mt * w:(mt + 1) * w], po[:])
                nc.sync.dma_start(
                    opart_d[:, c0:c0 + w].rearrange("(mt p) c -> p mt c", p=128),
                    ot[:].rearrange("p (mt c) -> p mt c", c=w))

            def attention_qb(qb, slots):
                nkt = 4 * (qb + 1)
                av = psAV.tile([128, 2 * AVS - 252], F32, tag="av",
                               name=f"av{qb}")
                # slot s, qtile j lives at av[:, s*AVS + j*65 : +65]
                avT = {}
                stage2 = []  # (s, j) divisions done, transpose pending

                def finish_stage1(s, j):
                    # reciprocal of the sums column, divide (DVE/Pool)
                    base = s * AVS + j * 65
                    rec = avsb.tile([128, 1], F32, tag="rec",
                                    name=f"rec{qb}_{s}_{j}")
                    with nc.allow_low_precision(reason="softmax recip"):
                        nc.vector.reciprocal(rec[:], av[:, base + 64:base + 65])
                    asb = avsb.tile([128, 64], BF16, tag="asb",
                                    name=f"asb{qb}_{s}_{j}")
                    # Pool cannot read PSUM on trn2 -> divide on DVE
                    nc.vector.tensor_scalar(asb[:], av[:, base:base + 64],
                                            rec[:, 0:1], None,
                                            op0=AluOpType.mult)
                    stage2.append((s, j, asb))

                def finish_stage2():
                    # transpose divided scores tiles back to [dh, q] (PE)
                    for s, j, asb in stage2:
                        if j % 2 == 0:
                            avT[s] = psJ.tile([64, 256], BF16, tag="pj",
                                              name=f"avT{qb}_{s}_{j}")
                        nc.tensor.transpose(
                            avT[s][:, (j % 2) * 128:(j % 2) * 128 + 128],
                            asb[:], eye[:])
                        if j % 2 == 1:
                            qt0 = 4 * qb + j - 1
                            nc.vector.tensor_copy(
                                at_all[s * 64:(s + 1) * 64,
                                       qt0 * KT:(qt0 + 2) * KT],
                                avT[s][:])
                    stage2.clear()

                def emit_avs_slot(ex_, ta_, s_):
                    for h_ in range(2):
                        t_ = ta_ + h_
                        for j_ in range(4):
                            qt_ = 4 * qb + j_
                            if t_ > qt_:
                                continue
                            # start=True clears has_written for the WHOLE
                            # PSUM bank on hw, so only the first matmul into
                            # this slot's bank may set it; later slices'
                            # first writes overwrite via the cleared bits
                            nc.tensor.matmul(
                                av[:, s_ * AVS + j_ * 65:
                                   s_ * AVS + j_ * 65 + 65],
                                ex_[:, h_ * QB + j_ * 128:
                                    h_ * QB + j_ * 128 + 128],
                                vp_all[:, t_ * VPW + s_ * 65:
                                       t_ * VPW + s_ * 65 + 65],
                                start=(t_ == 0 and j_ == 0),
                                stop=(t_ == qt_))

                def stage1_checks(ta_):
                    if ta_ + 1 >= 4 * qb:
                        for j_ in (ta_ - 4 * qb, ta_ + 1 - 4 * qb):
                            if 0 <= j_ < 4:
                                for s_ in slots:
                                    finish_stage1(s_, j_)

                def emit_avs(exs_, ta_):
                    for s_ in slots:
                        emit_avs_slot(exs_[s_], ta_, s_)
                    stage1_checks(ta_)

                prev = None
                for g in range(nkt // 2):
                    ta = 2 * g
                    # force-drain steps whose chunk this group's logits read
                    # (scan whole queue: NEVER-deadline entries may sit ahead);
                    # at qb start force the whole current chunk for slack
                    need = qb if g == 0 else (2 * g + 1) // 4
                    i = 0
                    while i < len(bg_steps):
                        if bg_steps[i][0] <= need:
                            bg_steps.pop(i)[1]()
                        else:
                            i += 1
                    exs = {}
                    for s in slots:
                        lg = psL.tile([128, 2 * QB], F32, tag="lg",
                                      name=f"lg{qb}_{g}_{s}")
                        ex = exp_pool.tile([128, 2 * QB], BF16, tag="ex",
                                           name=f"ex{qb}_{g}_{s}")
                        start_col = 0
                        for h in range(2):
                            t = ta + h
                            m = t - 4 * qb
                            off = 128 * m if m >= 0 else 0
                            base = h * QB
                            if h == 0:
                                start_col = off
                            nc.tensor.matmul(
                                lg[:, base + off:base + QB],
                                kt_rot[s * 64:(s + 1) * 64, t * KT:(t + 1) * KT],
                                qt_rot[s * 64:(s + 1) * 64,
                                       qb * QB + off:(qb + 1) * QB],
                                start=True, stop=(m < 0))
                            if m >= 0:
                                # -1e30 upper-triangle bias via eye @ M
                                nc.tensor.matmul(
                                    lg[:, base + off:base + off + 128],
                                    eye[:], maskm[:],
                                    start=False, stop=True)
                        m1_ = ta + 1 - 4 * qb
                        if 0 < start_col and m1_ > 1:
                            # second diagonal group: skip the fully-masked
                            # [QB : QB+128*m1_) junk columns with a split exp
                            nc.scalar.activation(ex[:, start_col:QB],
                                                 lg[:, start_col:QB],
                                                 AF.Exp, scale=0.125)
                            nc.scalar.activation(ex[:, QB + 128 * m1_:2 * QB],
                                                 lg[:, QB + 128 * m1_:2 * QB],
                                                 AF.Exp, scale=0.125)
                        else:
                            nc.scalar.activation(ex[:, start_col:2 * QB],
                                                 lg[:, start_col:2 * QB],
                                                 AF.Exp, scale=0.125)
                        exs[s] = ex
                        # AV for the previous group's same-slot scores is
                        # ready now — fills PE while ACT runs this slot's exp
                        # and the next slot's exp dependency clears
                        if prev is not None:
                            emit_avs_slot(prev[0][s], prev[1], s)
                    # fill PE behind the just-emitted logits while ACT works
                    if bg_steps:
                        best = min(range(len(bg_steps)),
                                   key=lambda i_: bg_steps[i_][0])
                        bg_steps.pop(best)[1]()
                    finish_stage2()
                    if prev is not None:
                        stage1_checks(prev[1])
                    prev = (exs, ta)
                emit_avs(*prev)
                if qb == NQB - 1:
                    # tail: interleave the final O-projection with the last
                    # qtiles' division/transpose chains
                    half1 = [e for e in stage2 if e[1] < 2]
                    half2 = [e for e in stage2 if e[1] >= 2]
                    stage2[:] = half1
                    finish_stage2()
                    emit_oproj_span(qb, 0, 2, "a")
                    stage2[:] = half2
                    finish_stage2()
                    emit_oproj_span(qb, 2, 4, "b")
                else:
                    finish_stage2()
                    queue_oproj(qb)

            # ---------------- interleaved main loop ---------------------
            def queue_proj_chunk(c):
                """DMA the chunk now; queue K/Q/V proj as background steps."""
                c0, c1 = c * CH, (c + 1) * CH
                xc_all = xcp.tile([128, 6 * CH], BF16, tag="xc", name=f"xca{c}",
                                  bufs=3)
                nc.sync.dma_start(
                    xc_all[:].rearrange("p (i c) -> p i c", c=CH),
                    xt_d[:, c0:c1].rearrange("(i p) c -> p i c", p=128))
                cosf_c = rtp.tile([128, CH], F32, tag="cosc", name=f"cosc{c}",
                                  bufs=3)
                sins_c = rtp.tile([128, CH], F32, tag="sinc", name=f"sinc{c}",
                                  bufs=3)
                nc.sync.dma_start(cosf_c[:], cosf_d[:, c0:c1])
                nc.sync.dma_start(sins_c[:], sins_d[:, c0:c1])
                xc = [xc_all[:, i * CH:(i + 1) * CH] for i in range(6)]

                def rope_step(w, dst):
                    ps = psJ.tile([128, CH], F32, tag="pj", name=f"pp{c}")
                    for i in range(6):
                        nc.tensor.matmul(ps[:], w[i], xc[i],
                                         start=(i == 0), stop=(i == 5))
                    tsw = rtp.tile([128, CH], F32, tag="tsw")
                    nc.vector.stream_shuffle(tsw[:], ps[:], ROT16)
                    m1 = rtp.tile([128, CH], F32, tag="m1")
                    nc.vector.tensor_tensor(m1[:], ps[:], cosf_c[:],
                                            op=AluOpType.mult)
                    m2 = rtp.tile([128, CH], F32, tag="m2")
                    nc.gpsimd.tensor_tensor(m2[:], tsw[:], sins_c[:],
                                            op=AluOpType.mult)
                    # DVE add in the front (shorter chain to first logits),
                    # Pool later (keeps DVE free in steady state)
                    eng = nc.vector if c <= 1 else nc.gpsimd
                    eng.tensor_tensor(dst[:, c0:c1], m1[:], m2[:],
                                      op=AluOpType.add)

                def v_step():
                    ps = psJ.tile([128, CH], F32, tag="pj", name=f"pv{c}")
                    for i in range(6):
                        nc.tensor.matmul(ps[:], wv_t[i], xc[i],
                                         start=(i == 0), stop=(i == 5))
                    vt = rtp.tile([128, CH], BF16, tag="vt")
                    nc.vector.tensor_copy(vt[:], ps[:])
                    for i_ in range(4):
                        t_ = 4 * c + i_
                        vtp = psJ.tile([128, 128], BF16, tag="pj",
                                       name=f"vtp{c}_{i_}")
                        nc.tensor.transpose(vtp[:],
                                            vt[:, i_ * 128:(i_ + 1) * 128],
                                            eye[:])
                        nc.vector.tensor_copy(
                            vp_all[:, t_ * VPW:t_ * VPW + 64], vtp[:, 0:64])
                        nc.vector.tensor_copy(
                            vp_all[:, t_ * VPW + 65:t_ * VPW + 129],
                            vtp[:, 64:128])

                bg_steps.append((c, lambda: rope_step(wk_t, kt_rot)))
                bg_steps.append((c, lambda: rope_step(wq_t, qt_rot)))
                bg_steps.append((c, v_step))

            def drain_bg():
                while bg_steps:
                    bg_steps.pop(0)[1]()

            queue_proj_chunk(0)
            init_consts_late()
            # drain chunk 0 now; later chunks prefetch 2 ahead and their
            # compute interleaves into the attention groups
            i0 = 0
            while i0 < len(bg_steps):
                if bg_steps[i0][0] <= 0:
                    bg_steps.pop(i0)[1]()
                else:
                    i0 += 1
            queue_proj_chunk(1)
            for qb in range(NQB):
                if qb + 2 < NCH:
                    queue_proj_chunk(qb + 2)
                attention_qb(qb, [0] + ([1] if qb in bset else []))
            drain_bg()
            if DEBUG:
                nc.sync.dma_start(dbg_kt[:], kt_rot[:])
                nc.sync.dma_start(dbg_qt[:], qt_rot[:])
                nc.sync.dma_start(dbg_vp[:], vp_all[:])
                nc.sync.dma_start(dbg_at[:], at_all[:])

    nc.compile()
    return nc


def _get_program(bset):
    key = tuple(bset)
    if key not in _PROGRAMS:
        _PROGRAMS[key] = _build_program(key)
    return _PROGRAMS[key]


def _to_bf16(a):
    import ml_dtypes
    return np.asarray(a, np.float32).astype(ml_dtypes.bfloat16)


def _prep_core_inputs(core, x2d_T16, token_positions, Wq, Wk, Wv, Wo):
    hA, hB = CORE_HEADS[core]
    pos = token_positions.astype(np.float64)
    inv_freq = 1.0 / (THETA ** (np.arange(0, DH, 2, dtype=np.float64) / DH))  # [32]
    ang = pos[:, None] * inv_freq[None, :]          # [S, 32]
    cosv, sinv = np.cos(ang), np.sin(ang)           # [S, 32]

    cosf = np.empty((128, S), np.float32)
    sins = np.empty((128, S), np.float32)
    for r in range(64):
        q, i = r // 32, r % 32
        f = 16 * q + (i % 16)
        cosf[r] = cosf[r + 64] = cosv[:, f].astype(np.float32)
        sgn = -1.0 if i < 16 else 1.0
        sins[r] = sins[r + 64] = (sgn * sinv[:, f]).astype(np.float32)

    rows = np.concatenate([hA * DH + PERM64, hB * DH + PERM64])
    wqt = _to_bf16(np.ascontiguousarray(Wq[rows].T))   # [768,128]
    wkt = _to_bf16(np.ascontiguousarray(Wk[rows].T))
    vrows = np.concatenate([np.arange(hA * DH, (hA + 1) * DH),
                            np.arange(hB * DH, (hB + 1) * DH)])
    wvt = _to_bf16(np.ascontiguousarray(Wv[vrows].T))  # [768,128]
    wot = _to_bf16(np.ascontiguousarray(Wo[:, vrows].T))  # [128,768]

    # -200 (not -inf): exp(-200*0.125) ~ 1e-11 == 0 for our sums, and the
    # hardware ACT exp table NaNs on astronomically negative inputs
    maskm = np.where(np.arange(128)[None, :] >= np.arange(128)[:, None],
                     0.0, -200.0).astype(np.float32)  # [k', q']
    return {
        "xt": x2d_T16,
        "wqt": wqt, "wkt": wkt, "wvt": wvt, "wot": wot,
        "cosf": cosf, "sins": sins,
        "maskm": _to_bf16(maskm),
        "eye": _to_bf16(np.eye(128, dtype=np.float32)),
    }


def _dispatch_group(nc, in_maps, devices):
    """Async-dispatch one program on a device subset; returns (arrs, names, avals, n)."""
    import jax
    from jax.sharding import Mesh, PartitionSpec
    from concourse import bass2jax, mybir

    bass2jax.install_neuronx_cc_hook()
    n = len(in_maps)
    partition_name = (nc.partition_id_tensor.name
                      if nc.partition_id_tensor else None)
    in_names, out_names, out_avals, zero_outs = [], [], [], []
    for alloc in nc.m.functions[0].allocations:
        if not isinstance(alloc, mybir.MemoryLocationSet):
            continue
        name = alloc.memorylocations[0].name
        if alloc.kind == "ExternalInput":
            if name != partition_name:
                in_names.append(name)
        elif alloc.kind == "ExternalOutput":
            shape = tuple(alloc.tensor_shape)
            dtype = mybir.dt.np(alloc.dtype)
            out_names.append(name)
            out_avals.append(jax.core.ShapedArray(shape, dtype))
            zero_outs.append(np.zeros(shape, dtype))
    n_params = len(in_names)
    all_names = in_names + out_names
    if partition_name is not None:
        all_names = all_names + [partition_name]
    donate = tuple(range(n_params, n_params + len(out_names)))

    def _body(*args):
        operands = list(args)
        if partition_name is not None:
            operands.append(bass2jax.partition_id_tensor())
        outs = bass2jax._bass_exec_p.bind(
            *operands, out_avals=tuple(out_avals), in_names=tuple(all_names),
            out_names=tuple(out_names), lowering_input_output_aliases=(),
            sim_require_finite=True, sim_require_nnan=True, nc=nc)
        return tuple(outs)

    try:
        from jax.experimental.shard_map import shard_map
    except ImportError:
        from jax.shard_map import shard_map  # newer jax

    mesh = Mesh(np.asarray(devices), ("core",))
    in_specs = (PartitionSpec("core"),) * (n_params + len(out_names))
    out_specs = (PartitionSpec("core"),) * len(out_names)
    sharded = jax.jit(
        shard_map(_body, mesh=mesh, in_specs=in_specs, out_specs=out_specs,
                  check_rep=False),
        donate_argnums=donate, keep_unused=True)
    per_core = [[np.asarray(m[nm]) for nm in in_names] for m in in_maps]
    concat_in = [np.concatenate([per_core[c][i] for c in range(n)], axis=0)
                 for i in range(n_params)]
    concat_zeros = [np.zeros((n * z.shape[0], *z.shape[1:]), z.dtype)
                    for z in zero_outs]
    out_arrs = sharded(*concat_in, *concat_zeros)
    return out_arrs, out_names, out_avals, n


def kernel(x, token_positions, Wq, Wk, Wv, Wo):
    import jax

    x = np.asarray(x)
    token_positions = np.asarray(token_positions)
    Wq, Wk, Wv, Wo = (np.asarray(a, np.float32) for a in (Wq, Wk, Wv, Wo))
    B = x.shape[0]
    assert x.shape == (B, S, D) and B == 1

    x2d_T16 = _to_bf16(np.ascontiguousarray(x[0].T))  # [768, 4096] bf16

    in_maps = [_prep_core_inputs(c, x2d_T16, token_positions, Wq, Wk, Wv, Wo)
               for c in range(8)]

    nc_even = _get_program(BSET_EVEN)
    nc_odd = _get_program(BSET_ODD)

    devs = jax.devices()
    # even program on devices 0-3 <- logical cores 0,2,4,6
    # odd  program on devices 4-7 <- logical cores 1,3,5,7
    g1_maps = [in_maps[c] for c in (0, 2, 4, 6)]
    g2_maps = [in_maps[c] for c in (1, 3, 5, 7)]

    arrs1, names1, avals1, n1 = _dispatch_group(nc_even, g1_maps, devs[0:4])
    arrs2, names2, avals2, n2 = _dispatch_group(nc_odd, g2_maps, devs[4:8])

    def collect(arrs, names, avals, n):
        res = []
        for c in range(n):
            res.append({
                nm: np.asarray(arrs[i]).reshape(n, *avals[i].shape)[c]
                for i, nm in enumerate(names)})
        return res

    res1 = collect(arrs1, names1, avals1, n1)
    res2 = collect(arrs2, names2, avals2, n2)

    acc = np.zeros((D, S), np.float32)
    for r in res1 + res2:
        acc += r["opart"].astype(np.float32)
    out = np.ascontiguousarray(acc.T).reshape(1, S, D)
    return out
